# revision 8
# baseline (speedup 1.0000x reference)
"""Trainium2 Bass kernel for nn_Attention_16441134809282 (sparse sliding-window GQA).

Self-contained: hardcodes shapes from the problem spec.
Sharding: 8 cores; core c owns q-heads {2c, 2c+1} and kv-head c (tensor
parallel over heads). Each core computes a partial output [T, D] (its heads'
contribution through w_out); the host sums the 8 partials.

Matmuls run in float32r (TRN2 fast fp32 path, 1 cyc/row at free-dim >= 256)
with fp32 PSUM accumulation; the value path (V, exp-probs) runs in bf16.

V1 schedule (vs. prior baseline at ~435 us):
- phase 1 warm start: q-proj runs one tb ahead of kv-proj so the wkv weight
  DMA wait hides under q matmuls; wq group 0 rides the sync ring first so
  the first matmul starts ~5 us earlier.
- phase 2: AV flush lag 2 (deeper PE/ACT pipeline), full tiles before masked
  tiles per j (mask DMA prefetch window), mask loads prefetched one j ahead,
  the per-head rep matmul deferred past the next head's first QK, and a
  global lazy out-projection unit queue (units pop inside attention, double
  on unmasked tiles, drain with ACT copies).
- per-unit [128,512] output stores (earlier final DMA).
"""
import os
from collections import deque

import ml_dtypes
import numpy as np

import concourse.bass as bass  # noqa: F401
import concourse.mybir as mybir
import concourse.tile as tile
from concourse import bacc
from concourse.bass_utils import run_bass_kernel_spmd
from concourse.masks import make_identity

# problem constants
B, T, D = 1, 2048, 3072
N, K, H = 16, 8, 256
G = N // K
SOFT_CAP = 50.0
WINDOW = 1024
ROPE_BASE = 10000.0
ROPE_SCALE = 1.0
K_MASK = -2.3819763e38
EPS = 1e-6

NCORES = 8
TB = T // 128       # 16 t-blocks
DC = D // 128       # 24 d-chunks (contraction)
JQ = T // 512       # 4 query chunks of 512
DCH = D // 512      # 6 output d-chunks of 512

F32 = mybir.dt.float32
F32R = mybir.dt.float32r
BF16 = mybir.dt.bfloat16
AF = mybir.ActivationFunctionType
ALU = mybir.AluOpType

_PROG_CACHE: dict = {}


def _build_program(band_key, band, debug=False):
    """band: list (len JQ) of list of (kb, mask_slot or None)."""
    n_masks = max(1, sum(1 for row in band for (_, m) in row if m is not None))
    nc = bacc.Bacc("TRN2", target_bir_lowering=False, debug=False, num_devices=NCORES)

    xt_e = nc.dram_tensor("xt", [TB, 128, DC, 128], F32R, kind="ExternalInput").ap()
    wq_e = nc.dram_tensor("wq", [DC // 4, 128, 4, 512], F32R, kind="ExternalInput").ap()
    wkv_e = nc.dram_tensor("wkv", [DC // 4, 128, 4, 512], F32R, kind="ExternalInput").ap()
    wo_e = nc.dram_tensor("wo", [DCH, 128, 4, 512], F32R, kind="ExternalInput").ap()
    tabs_e = nc.dram_tensor("tabs", [TB, 128, 8, 128], F32, kind="ExternalInput").ap()
    masks_e = nc.dram_tensor("masks", [128, n_masks, 512], F32, kind="ExternalInput").ap()
    onec_e = nc.dram_tensor("onec", [128, 1], BF16, kind="ExternalInput").ap()
    oner_e = nc.dram_tensor("oner", [1, 128], F32R, kind="ExternalInput").ap()
    out_e = nc.dram_tensor("out", [T, D], F32, kind="ExternalOutput").ap()

    with tile.TileContext(nc) as tc:
        with (
            tc.tile_pool(name="pers", bufs=1) as pers,
            # logits psum lives at banks 0-1, below phase 1's pools, so the
            # first phase-2 QK matmuls have no WAR wait on phase-1 psum release
            tc.tile_pool(name="plg", bufs=2, space="PSUM") as plgp,
        ):
            # persistent SBUF: transposed Q/K (f32r), natural V (bf16)
            QT = pers.tile([128, 4 * TB, 128], F32R)   # chunk = tb*4 + head*2 + hc
            KT = pers.tile([128, 2 * TB, 128], F32R)   # chunk = tb*2 + hc
            V = pers.tile([128, TB, 256], BF16)        # [t%128, tb, h]

            QTv = QT[:].rearrange("p (tb hh) f -> p hh tb f", hh=4)
            KTv = KT[:].rearrange("p (tb hc) f -> p hc tb f", hc=2)

            # ---------------- Phase 1: projections + RMSNorm + RoPE + transpose
            with (
                tc.tile_pool(name="wts", bufs=1) as wts,
                tc.tile_pool(name="xs", bufs=3) as xsp,
                tc.tile_pool(name="tab", bufs=2) as tabp,
                tc.tile_pool(name="rot", bufs=2) as rotp,
                tc.tile_pool(name="wk", bufs=1) as wk,
                tc.tile_pool(name="psq", bufs=2, space="PSUM") as psqp,
                tc.tile_pool(name="pskv", bufs=2, space="PSUM") as pskvp,
                tc.tile_pool(name="ptrq", bufs=1, space="PSUM") as ptrqp,
                tc.tile_pool(name="ptrk", bufs=1, space="PSUM") as ptrkp,
            ):
                wq_c = [wts.tile([128, 4, 512], F32R, tag=f"wq{g}", name=f"wq{g}")
                        for g in range(DC // 4)]
                wkv_c = [wts.tile([128, 4, 512], F32R, tag=f"wkv{g}", name=f"wkv{g}")
                         for g in range(DC // 4)]

                xs_t, tabq_t, tabk_t = {}, {}, {}

                def load_x(tb):
                    xs = xsp.tile([128, DC, 128], F32R, tag="xs", name=f"xs{tb}")
                    nc.sync.dma_start(xs[:, 0:DC // 2, :], xt_e[tb, :, 0:DC // 2, :])
                    nc.sync.dma_start(xs[:, DC // 2:, :], xt_e[tb, :, DC // 2:, :])
                    tq = tabp.tile([128, 4, 128], F32, tag="tabq", name=f"tabq{tb}")
                    nc.sync.dma_start(tq[:], tabs_e[tb, :, 0:4, :])
                    xs_t[tb], tabq_t[tb] = xs, tq

                def load_tabk(tb):
                    # scalar ring: it drains after wq1..5, well before the
                    # sync ring works through the wkv groups + xs strips
                    tk = tabp.tile([128, 4, 128], F32, tag="tabk", name=f"tabk{tb}")
                    nc.scalar.dma_start(tk[:], tabs_e[tb, :, 4:8, :])
                    tabk_t[tb] = tk

                # DMA priority: wq0 first on the sync ring (earliest first
                # matmul), then tb0's x + q-tables, then wkv groups
                # interleaved with tb1's strips. wq1..5 stream on the scalar
                # ring concurrently.
                nc.sync.dma_start(wq_c[0][:], wq_e[0])
                for g in range(1, DC // 4):
                    nc.scalar.dma_start(wq_c[g][:], wq_e[g])
                load_x(0)
                for g in range(3):
                    nc.sync.dma_start(wkv_c[g][:], wkv_e[g])
                load_x(1)
                for g in range(3, DC // 4):
                    nc.sync.dma_start(wkv_c[g][:], wkv_e[g])

                ident = wts.tile([128, 128], F32)
                make_identity(nc, ident[:])
                eps_t = wts.tile([128, 1], F32)
                nc.gpsimd.memset(eps_t[:], EPS)
                # touch ACT tables once now so loads don't land on the
                # phase-2 critical path
                warm = wts.tile([128, 1], F32)
                nc.scalar.activation(warm[:], eps_t[:], AF.Tanh)
                nc.scalar.activation(warm[:], warm[:], AF.Exp)

                def rsqrt_of_meansq(src_ap, nfree, tag):
                    """rs = rsqrt(mean(src^2) + EPS), per partition row."""
                    scr = wk.tile([128, nfree], F32, tag="sq_scr")
                    ssq = wk.tile([128, 1], F32, tag=tag + "_ssq")
                    nc.scalar.activation(scr[:], src_ap, AF.Square, accum_out=ssq[:])
                    sd = wk.tile([128, 1], F32, tag=tag + "_sd")
                    nc.scalar.activation(sd[:], ssq[:], AF.Sqrt,
                                         scale=1.0 / nfree, bias=eps_t[:])
                    rs = wk.tile([128, 1], F32, tag=tag + "_rs")
                    nc.vector.reciprocal(rs[:], sd[:])
                    return rs

                def rope_norm(dst, psrc, off, rs, tabs):
                    """dst[:, off:off+256] = rope(psrc[:, off:off+256] * rs), with
                    (1+scale) folded into tabs[0..3]."""
                    f = psrc[:, off:off + 128]
                    s = psrc[:, off + 128:off + 256]
                    dst_f = dst[:, off:off + 128]
                    dst_s = dst[:, off + 128:off + 256]
                    t2 = wk.tile([128, 128], F32, tag="rope_t2", bufs=2)
                    nc.vector.scalar_tensor_tensor(
                        dst_f, f, rs[:], tabs[:, 0, :], ALU.mult, ALU.mult)
                    nc.vector.scalar_tensor_tensor(
                        t2[:], s, rs[:], tabs[:, 1, :], ALU.mult, ALU.mult)
                    nc.vector.tensor_sub(dst_f, dst_f, t2[:])
                    nc.vector.scalar_tensor_tensor(
                        dst_s, s, rs[:], tabs[:, 2, :], ALU.mult, ALU.mult)
                    nc.vector.scalar_tensor_tensor(
                        t2[:], f, rs[:], tabs[:, 3, :], ALU.mult, ALU.mult)
                    nc.vector.tensor_add(dst_s, dst_s, t2[:])

                qrot_t = {}
                pend = None  # (tb, qrot, krot) awaiting transpose

                def do_q(tb):
                    if tb + 1 < TB and tb + 1 not in xs_t:
                        load_x(tb + 1)
                    xs = xs_t[tb]
                    psq = psqp.tile([128, 512], F32, tag="psq")
                    for dc in range(DC):
                        nc.tensor.matmul(psq[:], xs[:, dc, :],
                                         wq_c[dc // 4][:, dc % 4, :],
                                         start=(dc == 0), stop=(dc == DC - 1))
                    qrot = rotp.tile([128, 512], F32, tag="qrot", name=f"qrot{tb}")
                    for head in range(2):
                        rs = rsqrt_of_meansq(psq[:, head * 256:(head + 1) * 256],
                                             256, f"q{head}")
                        rope_norm(qrot, psq, head * 256, rs, tabq_t[tb])
                    qrot_t[tb] = qrot

                def do_kv(tb):
                    nonlocal pend
                    load_tabk(tb)
                    if pend is not None:
                        _emit_transposes(nc, tc, pend, ident, QT, KT,
                                         ptrqp, ptrkp)
                    xs = xs_t[tb]
                    pskv = pskvp.tile([128, 512], F32, tag="pskv")
                    for dc in range(DC):
                        nc.tensor.matmul(pskv[:], xs[:, dc, :],
                                         wkv_c[dc // 4][:, dc % 4, :],
                                         start=(dc == 0), stop=(dc == DC - 1))
                    krot = rotp.tile([128, 256], F32, tag="krot", name=f"krot{tb}")
                    rs = rsqrt_of_meansq(pskv[:, 0:256], 256, "k")
                    rope_norm(krot, pskv, 0, rs, tabk_t[tb])
                    rs = rsqrt_of_meansq(pskv[:, 256:512], 256, "v")
                    nc.vector.tensor_scalar_mul(V[:, tb, :], pskv[:, 256:512], rs[:])
                    pend = (tb, qrot_t[tb], krot)

                # warm-start order: q one tb ahead of kv
                do_q(0)
                do_q(1)
                for t in range(TB - 1):
                    do_kv(t)
                    if t + 2 < TB:
                        do_q(t + 2)
                do_kv(TB - 1)
                _emit_transposes(nc, tc, pend, ident, QT, KT, ptrqp, ptrkp)

            # ---------------- Phase 2+3: attention + output projection
            with tc.tile_pool(name="enc", bufs=1) as encpool:
                ENC = encpool.tile([128, 16, 512], F32R)  # chunk = head*8+hc*4+j
                ones_c = encpool.tile([128, 1], BF16)
                ones_r = encpool.tile([1, 128], F32R)
                nc.sync.dma_start(ones_c[:], onec_e[:])
                nc.sync.dma_start(ones_r[:], oner_e[:])
                wo_t = [encpool.tile([128, 4, 512], F32R, name=f"wo{dch}")
                        for dch in range(DCH)]
                _phase2(nc, tc, band, QTv, KTv, V, ENC, ones_c, ones_r, masks_e,
                        wo_t, wo_e, out_e, plgp)

    nc.compile()
    return nc


def _phase2(nc, tc, band, QTv, KTv, V, ENC, ones_c, ones_r, masks_e,
            wo_t, wo_e, out_e, plgp):
    with (
        tc.tile_pool(name="mks", bufs=2) as mkp,
        tc.tile_pool(name="act", bufs=2) as actp,
        tc.tile_pool(name="sml", bufs=2) as sml,
        tc.tile_pool(name="ost", bufs=4) as ostp,
        tc.tile_pool(name="pen", bufs=1, space="PSUM") as penp,
        tc.tile_pool(name="pdn", bufs=1, space="PSUM") as pdnp,
        tc.tile_pool(name="prp", bufs=1, space="PSUM") as prpp,
        tc.tile_pool(name="pso", bufs=2, space="PSUM") as psop,
    ):
        # per-j kb order: full tiles first (masked tiles get their DMA window)
        rows = [sorted(band[j], key=lambda t: t[1] is not None)
                for j in range(JQ)]
        mtiles = {}  # j -> {mslot: ap}

        def load_masks(j):
            mslots = sorted(m for (_, m) in band[j] if m is not None)
            if not mslots:
                mtiles[j] = {}
                return
            m0, nm = mslots[0], len(mslots)
            assert mslots == list(range(m0, m0 + nm))
            mk = mkp.tile([128, nm, 512], F32, tag="mk", name=f"mkj{j}")
            nc.sync.dma_start(mk[:], masks_e[:, m0:m0 + nm, :])
            mtiles[j] = {m: mk[:, i, :] for i, m in enumerate(mslots)}

        load_masks(0)
        load_masks(1)

        pending = deque()
        drain_mode = False

        def unit(j, r, dch):
            def emit():
                stage = ostp.tile([128, 512], F32, tag="stage",
                                  name=f"st{j}_{r}_{dch}")
                po = psop.tile([128, 512], F32, tag="po")
                for hh in range(4):
                    head, hc = hh >> 1, hh & 1
                    nc.tensor.matmul(
                        po[:],
                        ENC[:, head * 8 + hc * 4 + j, r * 128:(r + 1) * 128],
                        wo_t[dch][:, hh, :],
                        start=(hh == 0), stop=(hh == 3))
                if drain_mode:
                    nc.scalar.activation(stage[:], po[:], AF.Copy)
                else:
                    nc.vector.tensor_copy(stage[:], po[:])
                tb = 4 * j + r
                nc.sync.dma_start(
                    out_e[tb * 128:(tb + 1) * 128, dch * 512:(dch + 1) * 512],
                    stage[:])
            return emit

        fin_prev = None  # deferred per-head finalize (rep matmul + folds)
        first = True
        for j in range(JQ):
            kbs = rows[j]
            nkb = len(kbs)
            for head in range(2):
                enc_ps = penp.tile([128, 2, 512], F32, tag="enc")
                den_ps = pdnp.tile([1, 512], F32, tag="den")
                pend_av = deque()

                def flush_av():
                    i, kb, ex = pend_av.popleft()
                    nc.tensor.matmul(den_ps[:], ones_c[:], ex[:],
                                     start=(i == 0), stop=(i == nkb - 1))
                    for hc in range(2):
                        nc.tensor.matmul(
                            enc_ps[:, hc, :],
                            V[:, kb, hc * 128:(hc + 1) * 128], ex[:],
                            start=(i == 0), stop=(i == nkb - 1))
                    if pending:
                        pending.popleft()()
                    if kbs[i][1] is None and pending:
                        pending.popleft()()

                for i, (kb, mslot) in enumerate(kbs):
                    lg = plgp.tile([128, 512], F32, tag="lg")
                    for hc in range(2):
                        nc.tensor.matmul(
                            lg[:], KTv[:, hc, kb, :],
                            QTv[:, head * 2 + hc, 4 * j:4 * j + 4, :],
                            start=(hc == 0), stop=(hc == 1))
                    th = actp.tile([128, 512], F32, tag="th")
                    nc.scalar.activation(th[:], lg[:], AF.Tanh,
                                         scale=1.0 / SOFT_CAP)
                    ex = actp.tile([128, 512], BF16, tag="ex", bufs=3)
                    if mslot is not None:
                        nc.vector.scalar_tensor_tensor(
                            th[:], th[:], SOFT_CAP, mtiles[j][mslot],
                            ALU.mult, ALU.add)
                        nc.scalar.activation(ex[:], th[:], AF.Exp)
                    else:
                        nc.scalar.activation(ex[:], th[:], AF.Exp,
                                             scale=SOFT_CAP)
                    if i == 0 and fin_prev is not None:
                        fin_prev()
                        fin_prev = None
                    pend_av.append((i, kb, ex))
                    if len(pend_av) >= 3:
                        flush_av()
                while pend_av:
                    flush_av()

                # part A now (DVE copy of the denominator row); the PE rep
                # matmul + folds are deferred past the next head's first QK
                den_sb = sml.tile([1, 512], F32R, tag="den_sb")
                nc.vector.tensor_copy(den_sb[:], den_ps[:])

                def make_fin(j=j, head=head, den_sb=den_sb, enc_ps=enc_ps):
                    def fin():
                        rep_ps = prpp.tile([128, 512], F32, tag="rep")
                        nc.tensor.matmul(rep_ps[:], ones_r[:], den_sb[:],
                                         start=True, stop=True)
                        rep_rec = sml.tile([128, 512], F32, tag="rep_rec")
                        nc.vector.reciprocal_approx_fast(rep_rec[:], rep_ps[:])
                        for hc in range(2):
                            nc.vector.tensor_mul(
                                ENC[:, head * 8 + hc * 4 + j, :],
                                enc_ps[:, hc, :], rep_rec[:])
                        if head == 1:
                            pending.extend(unit(j, r, dch)
                                           for dch in range(DCH)
                                           for r in range(4))
                    return fin

                fin_prev = make_fin()

            if first:
                for dch in range(DCH):
                    nc.scalar.dma_start(wo_t[dch][:], wo_e[dch])
                first = False
            if j + 2 < JQ:
                load_masks(j + 2)

        fin_prev()
        drain_mode = True
        while pending:
            pending.popleft()()


def _emit_transposes(nc, tc, pend, ident, QT, KT, ptrqp, ptrkp):
    tb, qrot, krot = pend
    ptr = ptrqp.tile([128, 4, 128], F32, tag="ptrq")
    for c in range(4):
        nc.tensor.transpose(ptr[:, c, :], qrot[:, c * 128:(c + 1) * 128], ident[:])
    nc.scalar.activation(QT[:, tb * 4:tb * 4 + 4, :], ptr[:], AF.Copy)
    ptr2 = ptrkp.tile([128, 2, 128], F32, tag="ptrk")
    for c in range(2):
        nc.tensor.transpose(ptr2[:, c, :], krot[:, c * 128:(c + 1) * 128], ident[:])
    nc.vector.tensor_copy(KT[:, tb * 2:tb * 2 + 2, :], ptr2[:])


def _host_prepare(x, segment_pos, attn_mask, w_q, w_kv, w_out, q_scale, k_scale):
    x2 = np.ascontiguousarray(np.asarray(x, np.float32).reshape(T, D))
    pos = np.asarray(segment_pos).reshape(T).astype(np.int64)
    am = np.asarray(attn_mask).reshape(T, T).astype(bool)

    # rope tables, fp32 like the reference
    half = H // 2
    fraction = (2.0 * np.arange(half, dtype=np.float32) / np.float32(H)).astype(np.float32)
    timescale = (np.float32(ROPE_BASE) ** fraction).astype(np.float32)
    sinusoid = (pos.astype(np.float32)[:, None] / timescale[None, :]) / np.float32(ROPE_SCALE)
    sin = np.sin(sinusoid).astype(np.float32)
    cos = np.cos(sinusoid).astype(np.float32)
    qsf = (1.0 + np.asarray(q_scale, np.float32))
    ksf = (1.0 + np.asarray(k_scale, np.float32))
    # tabs[t, 0..7, i]: q: cos*qsf_f, sin*qsf_s, cos*qsf_s, sin*qsf_f; then k
    tabs = np.empty((T, 8, half), np.float32)
    tabs[:, 0] = cos * qsf[None, :half]
    tabs[:, 1] = sin * qsf[None, half:]
    tabs[:, 2] = cos * qsf[None, half:]
    tabs[:, 3] = sin * qsf[None, :half]
    tabs[:, 4] = cos * ksf[None, :half]
    tabs[:, 5] = sin * ksf[None, half:]
    tabs[:, 6] = cos * ksf[None, half:]
    tabs[:, 7] = sin * ksf[None, :half]
    tabs = np.ascontiguousarray(tabs.reshape(TB, 128, 8, half))

    # combined mask -> band structure + additive mask tiles (transposed [k, q])
    sliding = (pos[None, :] > pos[:, None] - WINDOW) & (pos[None, :] < pos[:, None] + WINDOW)
    comb = am & sliding
    band = []
    mask_list = []
    for j in range(JQ):
        row = []
        sub_q = comb[j * 512:(j + 1) * 512]
        for kb in range(T // 128):
            sub = sub_q[:, kb * 128:(kb + 1) * 128]
            if not sub.any():
                continue
            if sub.all():
                row.append((kb, None))
            else:
                mask_list.append(
                    np.where(sub.T, np.float32(0.0), np.float32(K_MASK)))
                row.append((kb, len(mask_list) - 1))
        band.append(row)
    masks = (np.ascontiguousarray(np.stack(mask_list, axis=1).astype(np.float32))
             if mask_list else np.zeros((128, 1, 512), np.float32))

    # x transposed + tiled: xt[tb, p, dc, t] = x2[tb*128+t, dc*128+p]
    xt = np.ascontiguousarray(
        x2.reshape(TB, 128, DC, 128).transpose(0, 3, 2, 1))

    return x2, xt, tabs, band, masks


def kernel(x, segment_pos, attn_mask, w_q, w_kv, w_out, q_scale, k_scale):
    x = np.asarray(x, np.float32)
    w_q = np.asarray(w_q, np.float32)
    w_kv = np.asarray(w_kv, np.float32)
    w_out = np.asarray(w_out, np.float32)
    assert x.shape == (B, T, D) and w_q.shape == (N, D, H)

    x2, xt, tabs, band, masks = _host_prepare(
        x, segment_pos, attn_mask, w_q, w_kv, w_out, q_scale, k_scale)

    band_key = tuple(tuple(row) for row in band)
    if band_key not in _PROG_CACHE:
        _PROG_CACHE[band_key] = _build_program(band_key, band)
    nc = _PROG_CACHE[band_key]

    in_maps = []
    for c in range(NCORES):
        wqc = np.concatenate([w_q[2 * c], w_q[2 * c + 1]], axis=1)  # [D, 512]
        wqc = np.ascontiguousarray(
            wqc.reshape(DC // 4, 4, 128, 512).transpose(0, 2, 1, 3))
        wkvc = np.concatenate([w_kv[0, c], w_kv[1, c]], axis=1)     # [D, 512]
        wkvc = np.ascontiguousarray(
            wkvc.reshape(DC // 4, 4, 128, 512).transpose(0, 2, 1, 3))
        # wo[dch, p, hh, n] = w_out[2c + head][hc*128 + p, dch*512 + n]
        woc = np.empty((DCH, 128, 4, 512), np.float32)
        for hh in range(4):
            head, hc = hh >> 1, hh & 1
            woc[:, :, hh, :] = w_out[2 * c + head][hc * 128:(hc + 1) * 128] \
                .reshape(128, DCH, 512).transpose(1, 0, 2)
        in_maps.append({
            "xt": xt, "wq": wqc, "wkv": wkvc, "wo": np.ascontiguousarray(woc),
            "tabs": tabs, "masks": masks,
            "onec": np.ones((128, 1), ml_dtypes.bfloat16),
            "oner": np.ones((1, 128), np.float32),
        })

    trace = bool(int(os.environ.get("BASS_ATTN_TRACE", "0")))
    res = run_bass_kernel_spmd(nc, in_maps, list(range(NCORES)), trace=trace)
    if trace and res.exec_time_ns is not None:
        print(f"HW exec time: {res.exec_time_ns} ns")
        kernel._last_exec_ns = res.exec_time_ns
        kernel._last_results = res

    total = np.zeros((T, D), np.float64)
    for c in range(NCORES):
        total += res.results[c]["out"].astype(np.float64)
    return total.astype(np.float32).reshape(B, T, D)


# revision 22
# speedup vs baseline: 1.0227x; 1.0227x over previous
"""Trainium2 Bass kernel for nn_Attention_16441134809282 (sparse sliding-window GQA).

Self-contained: hardcodes shapes from the problem spec.
Sharding: 8 cores; core c owns q-heads {2c, 2c+1} and kv-head c (tensor
parallel over heads). Each core computes a partial output [T, D] (its heads'
contribution through w_out); the host sums the 8 partials.

Matmuls run in float32r (TRN2 fast fp32 path, 1 cyc/row at free-dim >= 256)
with fp32 PSUM accumulation; the value path (V, exp-probs) runs in bf16.

V1 schedule (vs. prior baseline at ~435 us):
- phase 1 warm start: q-proj runs one tb ahead of kv-proj so the wkv weight
  DMA wait hides under q matmuls; wq group 0 rides the sync ring first so
  the first matmul starts ~5 us earlier.
- phase 2: AV flush lag 2 (deeper PE/ACT pipeline), full tiles before masked
  tiles per j (mask DMA prefetch window), mask loads prefetched one j ahead,
  the per-head rep matmul deferred past the next head's first QK, and a
  global lazy out-projection unit queue (units pop inside attention, double
  on unmasked tiles, drain with ACT copies).
- per-unit [128,512] output stores (earlier final DMA).
"""
import os
from collections import deque

import ml_dtypes
import numpy as np

import concourse.bass as bass  # noqa: F401
import concourse.mybir as mybir
import concourse.tile as tile
from concourse import bacc
from concourse.bass_utils import run_bass_kernel_spmd
from concourse.masks import make_identity

# problem constants
B, T, D = 1, 2048, 3072
N, K, H = 16, 8, 256
G = N // K
SOFT_CAP = 50.0
WINDOW = 1024
ROPE_BASE = 10000.0
ROPE_SCALE = 1.0
K_MASK = -2.3819763e38
EPS = 1e-6

NCORES = 8
TB = T // 128       # 16 t-blocks
DC = D // 128       # 24 d-chunks (contraction)
JQ = T // 512       # 4 query chunks of 512
DCH = D // 512      # 6 output d-chunks of 512

F32 = mybir.dt.float32
F32R = mybir.dt.float32r
BF16 = mybir.dt.bfloat16
AF = mybir.ActivationFunctionType
ALU = mybir.AluOpType

_PROG_CACHE: dict = {}


def _build_program(band_key, band, debug=False):
    """band: list (len JQ) of list of (kb, mask_slot or None)."""
    n_masks = max(1, sum(1 for row in band for (_, m) in row if m is not None))
    nc = bacc.Bacc("TRN2", target_bir_lowering=False, debug=False, num_devices=NCORES)

    xt_e = nc.dram_tensor("xt", [TB, 128, DC, 128], F32R, kind="ExternalInput").ap()
    wq_e = nc.dram_tensor("wq", [DC // 4, 128, 4, 512], F32R, kind="ExternalInput").ap()
    wkv_e = nc.dram_tensor("wkv", [DC // 4, 128, 4, 512], F32R, kind="ExternalInput").ap()
    wo_e = nc.dram_tensor("wo", [DCH, 128, 4, 512], F32R, kind="ExternalInput").ap()
    tabs_e = nc.dram_tensor("tabs", [TB, 128, 8, 128], F32, kind="ExternalInput").ap()
    masks_e = nc.dram_tensor("masks", [128, n_masks, 512], F32, kind="ExternalInput").ap()
    onec_e = nc.dram_tensor("onec", [128, 1], BF16, kind="ExternalInput").ap()
    oner_e = nc.dram_tensor("oner", [1, 128], F32R, kind="ExternalInput").ap()
    out_e = nc.dram_tensor("out", [T, D], F32, kind="ExternalOutput").ap()

    with tile.TileContext(nc) as tc:
        with (
            tc.tile_pool(name="pers", bufs=1) as pers,
            # logits psum lives at banks 0-1, below phase 1's pools, so the
            # first phase-2 QK matmuls have no WAR wait on phase-1 psum release
            tc.tile_pool(name="plg", bufs=2, space="PSUM") as plgp,
        ):
            # persistent SBUF: transposed Q/K (f32r), natural V (bf16)
            QT = pers.tile([128, 4 * TB, 128], F32R)   # chunk = tb*4 + head*2 + hc
            KT = pers.tile([128, 2 * TB, 128], F32R)   # chunk = tb*2 + hc
            V = pers.tile([128, TB, 256], BF16)        # [t%128, tb, h]

            QTv = QT[:].rearrange("p (tb hh) f -> p hh tb f", hh=4)
            KTv = KT[:].rearrange("p (tb hc) f -> p hc tb f", hc=2)

            # ---------------- Phase 1: projections + RMSNorm + RoPE + transpose
            with (
                tc.tile_pool(name="wts", bufs=1) as wts,
                tc.tile_pool(name="xs", bufs=3) as xsp,
                tc.tile_pool(name="tab", bufs=2) as tabp,
                tc.tile_pool(name="rot", bufs=2) as rotp,
                tc.tile_pool(name="wk", bufs=1) as wk,
                tc.tile_pool(name="psq", bufs=2, space="PSUM") as psqp,
                tc.tile_pool(name="pskv", bufs=2, space="PSUM") as pskvp,
                tc.tile_pool(name="ptrq", bufs=1, space="PSUM") as ptrqp,
                tc.tile_pool(name="ptrk", bufs=1, space="PSUM") as ptrkp,
            ):
                wq_c = [wts.tile([128, 4, 512], F32R, tag=f"wq{g}", name=f"wq{g}")
                        for g in range(DC // 4)]
                wkv_c = [wts.tile([128, 4, 512], F32R, tag=f"wkv{g}", name=f"wkv{g}")
                         for g in range(DC // 4)]

                xs_t, tabq_t, tabk_t = {}, {}, {}

                def load_x(tb):
                    xs = xsp.tile([128, DC, 128], F32R, tag="xs", name=f"xs{tb}")
                    nc.sync.dma_start(xs[:, 0:DC // 2, :], xt_e[tb, :, 0:DC // 2, :])
                    nc.sync.dma_start(xs[:, DC // 2:, :], xt_e[tb, :, DC // 2:, :])
                    xs_t[tb] = xs

                def load_tabq(tb):
                    tq = tabp.tile([128, 4, 128], F32, tag="tabq",
                                   name=f"tabq{tb}")
                    nc.sync.dma_start(tq[:], tabs_e[tb, :, 0:4, :])
                    tabq_t[tb] = tq

                def load_tabk(tb):
                    # scalar ring: it drains after wq1..5, well before the
                    # sync ring works through the wkv groups + xs strips
                    tk = tabp.tile([128, 4, 128], F32, tag="tabk", name=f"tabk{tb}")
                    nc.scalar.dma_start(tk[:], tabs_e[tb, :, 4:8, :])
                    tabk_t[tb] = tk

                # DMA priority: wq0 rides the sync ring first (earliest first
                # matmul); all remaining weight groups stream on the scalar
                # ring in consumption order while the sync ring carries the
                # x strips + q-tables.
                nc.sync.dma_start(wq_c[0][:], wq_e[0])
                for g in range(1, DC // 4):
                    nc.scalar.dma_start(wq_c[g][:], wq_e[g])
                for g in range(DC // 4):
                    nc.scalar.dma_start(wkv_c[g][:], wkv_e[g])
                load_x(0)
                load_x(1)
                load_x(2)
                load_tabq(0)
                load_tabq(1)

                ident = wts.tile([128, 128], F32)
                make_identity(nc, ident[:])
                eps_t = wts.tile([128, 1], F32)
                nc.gpsimd.memset(eps_t[:], EPS)
                # touch ACT tables once now so loads don't land on the
                # phase-2 critical path
                warm = wts.tile([128, 1], F32)
                nc.scalar.activation(warm[:], eps_t[:], AF.Tanh)
                nc.scalar.activation(warm[:], warm[:], AF.Exp)

                def rsqrt_of_meansq(src_ap, nfree, tag):
                    """rs = rsqrt(mean(src^2) + EPS), per partition row."""
                    scr = wk.tile([128, nfree], F32, tag="sq_scr")
                    ssq = wk.tile([128, 1], F32, tag=tag + "_ssq")
                    nc.scalar.activation(scr[:], src_ap, AF.Square, accum_out=ssq[:])
                    sd = wk.tile([128, 1], F32, tag=tag + "_sd")
                    nc.scalar.activation(sd[:], ssq[:], AF.Sqrt,
                                         scale=1.0 / nfree, bias=eps_t[:])
                    rs = wk.tile([128, 1], F32, tag=tag + "_rs")
                    nc.vector.reciprocal(rs[:], sd[:])
                    return rs

                def rope_norm(dst, psrc, off, rs, tabs):
                    """dst[:, off:off+256] = rope(psrc[:, off:off+256] * rs), with
                    (1+scale) folded into tabs[0..3]."""
                    f = psrc[:, off:off + 128]
                    s = psrc[:, off + 128:off + 256]
                    dst_f = dst[:, off:off + 128]
                    dst_s = dst[:, off + 128:off + 256]
                    t2 = wk.tile([128, 128], F32, tag="rope_t2", bufs=2)
                    nc.vector.scalar_tensor_tensor(
                        dst_f, f, rs[:], tabs[:, 0, :], ALU.mult, ALU.mult)
                    nc.vector.scalar_tensor_tensor(
                        t2[:], s, rs[:], tabs[:, 1, :], ALU.mult, ALU.mult)
                    nc.vector.tensor_sub(dst_f, dst_f, t2[:])
                    nc.vector.scalar_tensor_tensor(
                        dst_s, s, rs[:], tabs[:, 2, :], ALU.mult, ALU.mult)
                    nc.vector.scalar_tensor_tensor(
                        t2[:], f, rs[:], tabs[:, 3, :], ALU.mult, ALU.mult)
                    nc.vector.tensor_add(dst_s, dst_s, t2[:])

                qrot_t = {}
                pend = None  # (tb, qrot, krot) awaiting transpose

                def do_q(tb):
                    xs = xs_t[tb]
                    psq = psqp.tile([128, 512], F32, tag="psq")
                    for dc in range(DC):
                        nc.tensor.matmul(psq[:], xs[:, dc, :],
                                         wq_c[dc // 4][:, dc % 4, :],
                                         start=(dc == 0), stop=(dc == DC - 1))
                    qrot = rotp.tile([128, 512], F32, tag="qrot", bufs=3,
                                     name=f"qrot{tb}")
                    for head in range(2):
                        rs = rsqrt_of_meansq(psq[:, head * 256:(head + 1) * 256],
                                             256, f"q{head}")
                        rope_norm(qrot, psq, head * 256, rs, tabq_t[tb])
                    qrot_t[tb] = qrot
                    # tabq ring is 2 deep; tb's epilogue above was the last
                    # reader of the slot tb+2 recycles
                    if tb + 2 < TB:
                        load_tabq(tb + 2)

                def do_kv(tb):
                    nonlocal pend
                    load_tabk(tb)
                    if pend is not None:
                        _emit_transposes(nc, tc, pend, ident, QT, KT,
                                         ptrqp, ptrkp)
                    xs = xs_t[tb]
                    pskv = pskvp.tile([128, 512], F32, tag="pskv")
                    for dc in range(DC):
                        nc.tensor.matmul(pskv[:], xs[:, dc, :],
                                         wkv_c[dc // 4][:, dc % 4, :],
                                         start=(dc == 0), stop=(dc == DC - 1))
                    krot = rotp.tile([128, 256], F32, tag="krot", name=f"krot{tb}")
                    rs = rsqrt_of_meansq(pskv[:, 0:256], 256, "k")
                    rope_norm(krot, pskv, 0, rs, tabk_t[tb])
                    rs = rsqrt_of_meansq(pskv[:, 256:512], 256, "v")
                    nc.vector.tensor_scalar_mul(V[:, tb, :], pskv[:, 256:512], rs[:])
                    pend = (tb, qrot_t[tb], krot)
                    # prefetch the x strip this buffer slot frees up (kv(tb)'s
                    # matmuls above are the last readers of xs[tb], so the WAR
                    # on the recycled buffer is correct and already emitted)
                    if tb + 3 < TB:
                        load_x(tb + 3)

                # warm-start order: q runs ahead of kv so the wkv weight
                # stream (behind wq on the scalar ring) hides under q matmuls
                do_q(0)
                do_q(1)
                do_q(2)
                do_kv(0)
                do_kv(1)
                for t in range(3, TB):
                    do_q(t)
                    do_kv(t - 1)
                do_kv(TB - 1)
                _emit_transposes(nc, tc, pend, ident, QT, KT, ptrqp, ptrkp)

            # ---------------- Phase 2+3: attention + output projection
            with tc.tile_pool(name="enc", bufs=1) as encpool:
                ENC = encpool.tile([128, 16, 512], F32R)  # chunk = head*8+hc*4+j
                ones_c = encpool.tile([128, 1], BF16)
                ones_r = encpool.tile([1, 128], F32R)
                nc.sync.dma_start(ones_c[:], onec_e[:])
                nc.sync.dma_start(ones_r[:], oner_e[:])
                wo_t = [encpool.tile([128, 4, 512], F32R, name=f"wo{dch}")
                        for dch in range(DCH)]
                _phase2(nc, tc, band, QTv, KTv, V, ENC, ones_c, ones_r, masks_e,
                        wo_t, wo_e, out_e, plgp)

    nc.compile()
    return nc


def _phase2(nc, tc, band, QTv, KTv, V, ENC, ones_c, ones_r, masks_e,
            wo_t, wo_e, out_e, plgp):
    with (
        tc.tile_pool(name="mks", bufs=2) as mkp,
        tc.tile_pool(name="act", bufs=2) as actp,
        tc.tile_pool(name="sml", bufs=2) as sml,
        tc.tile_pool(name="ost", bufs=4) as ostp,
        tc.tile_pool(name="pen", bufs=1, space="PSUM") as penp,
        tc.tile_pool(name="pdn", bufs=1, space="PSUM") as pdnp,
        tc.tile_pool(name="prp", bufs=1, space="PSUM") as prpp,
        tc.tile_pool(name="pso", bufs=2, space="PSUM") as psop,
    ):
        # per-j kb order: full tiles first (masked tiles get their DMA window)
        rows = [sorted(band[j], key=lambda t: t[1] is not None)
                for j in range(JQ)]
        mtiles = {}  # j -> {mslot: ap}

        def load_masks(j):
            mslots = sorted(m for (_, m) in band[j] if m is not None)
            if not mslots:
                mtiles[j] = {}
                return
            m0, nm = mslots[0], len(mslots)
            assert mslots == list(range(m0, m0 + nm))
            mk = mkp.tile([128, nm, 512], F32, tag="mk", name=f"mkj{j}")
            nc.sync.dma_start(mk[:], masks_e[:, m0:m0 + nm, :])
            mtiles[j] = {m: mk[:, i, :] for i, m in enumerate(mslots)}

        jorder = [1, 0, 2, 3]
        load_masks(jorder[0])
        load_masks(jorder[1])

        pending = deque()
        drain_mode = False
        copy_tick = 0

        def unit(j, r, dch):
            def emit():
                nonlocal copy_tick
                stage = ostp.tile([128, 512], F32, tag="stage",
                                  name=f"st{j}_{r}_{dch}")
                po = psop.tile([128, 512], F32, tag="po")
                for hh in range(4):
                    head, hc = hh >> 1, hh & 1
                    nc.tensor.matmul(
                        po[:],
                        ENC[:, head * 8 + hc * 4 + j, r * 128:(r + 1) * 128],
                        wo_t[dch][:, hh, :],
                        start=(hh == 0), stop=(hh == 3))
                # alternate the psum-drain copy between ACT and DVE so neither
                # engine's queue gates the po psum reuse
                copy_tick += 1
                if drain_mode or copy_tick % 2:
                    nc.scalar.activation(stage[:], po[:], AF.Copy)
                else:
                    nc.vector.tensor_copy(stage[:], po[:])
                tb = 4 * j + r
                nc.sync.dma_start(
                    out_e[tb * 128:(tb + 1) * 128, dch * 512:(dch + 1) * 512],
                    stage[:])
            return emit

        fin_prev = None  # deferred per-head finalize (rep matmul + folds)
        first = True
        for jidx, j in enumerate(jorder):
            kbs = rows[j]
            nkb = len(kbs)
            for head in range(2):
                enc_ps = penp.tile([128, 2, 512], F32, tag="enc")
                den_ps = pdnp.tile([1, 512], F32, tag="den")
                pend_av = deque()

                def flush_av():
                    i, kb, ex = pend_av.popleft()
                    nc.tensor.matmul(den_ps[:], ones_c[:], ex[:],
                                     start=(i == 0), stop=(i == nkb - 1))
                    for hc in range(2):
                        nc.tensor.matmul(
                            enc_ps[:, hc, :],
                            V[:, kb, hc * 128:(hc + 1) * 128], ex[:],
                            start=(i == 0), stop=(i == nkb - 1))
                    if pending:
                        pending.popleft()()
                    if kbs[i][1] is None and pending:
                        pending.popleft()()

                for i, (kb, mslot) in enumerate(kbs):
                    lg = plgp.tile([128, 512], F32, tag="lg")
                    for hc in range(2):
                        nc.tensor.matmul(
                            lg[:], KTv[:, hc, kb, :],
                            QTv[:, head * 2 + hc, 4 * j:4 * j + 4, :],
                            start=(hc == 0), stop=(hc == 1))
                    th = actp.tile([128, 512], F32, tag="th")
                    nc.scalar.activation(th[:], lg[:], AF.Tanh,
                                         scale=1.0 / SOFT_CAP)
                    ex = actp.tile([128, 512], BF16, tag="ex", bufs=3)
                    if mslot is not None:
                        nc.vector.scalar_tensor_tensor(
                            th[:], th[:], SOFT_CAP, mtiles[j][mslot],
                            ALU.mult, ALU.add)
                        nc.scalar.activation(ex[:], th[:], AF.Exp)
                    else:
                        nc.scalar.activation(ex[:], th[:], AF.Exp,
                                             scale=SOFT_CAP)
                    if i == 0 and fin_prev is not None:
                        fin_prev()
                        fin_prev = None
                    pend_av.append((i, kb, ex))
                    if len(pend_av) >= 3:
                        flush_av()
                while pend_av:
                    flush_av()

                # part A now (DVE copy of the denominator row); the PE rep
                # matmul + folds are deferred past the next head's first QK
                den_sb = sml.tile([1, 512], F32R, tag="den_sb")
                nc.vector.tensor_copy(den_sb[:], den_ps[:])

                def make_fin(j=j, head=head, den_sb=den_sb, enc_ps=enc_ps):
                    def fin():
                        rep_ps = prpp.tile([128, 512], F32, tag="rep")
                        nc.tensor.matmul(rep_ps[:], ones_r[:], den_sb[:],
                                         start=True, stop=True)
                        rep_rec = sml.tile([128, 512], F32, tag="rep_rec")
                        nc.vector.reciprocal_approx_fast(rep_rec[:], rep_ps[:])
                        for hc in range(2):
                            nc.vector.tensor_mul(
                                ENC[:, head * 8 + hc * 4 + j, :],
                                enc_ps[:, hc, :], rep_rec[:])
                        if head == 1:
                            pending.extend(unit(j, r, dch)
                                           for dch in range(DCH)
                                           for r in range(4))
                    return fin

                fin_prev = make_fin()

            if first:
                for dch in range(DCH):
                    nc.scalar.dma_start(wo_t[dch][:], wo_e[dch])
                first = False
            if jidx + 2 < JQ:
                load_masks(jorder[jidx + 2])

        fin_prev()
        drain_mode = True
        while pending:
            pending.popleft()()


def _emit_transposes(nc, tc, pend, ident, QT, KT, ptrqp, ptrkp):
    tb, qrot, krot = pend
    ptr = ptrqp.tile([128, 4, 128], F32, tag="ptrq")
    for c in range(4):
        nc.tensor.transpose(ptr[:, c, :], qrot[:, c * 128:(c + 1) * 128], ident[:])
    nc.scalar.activation(QT[:, tb * 4:tb * 4 + 4, :], ptr[:], AF.Copy)
    ptr2 = ptrkp.tile([128, 2, 128], F32, tag="ptrk")
    for c in range(2):
        nc.tensor.transpose(ptr2[:, c, :], krot[:, c * 128:(c + 1) * 128], ident[:])
    nc.vector.tensor_copy(KT[:, tb * 2:tb * 2 + 2, :], ptr2[:])


def _host_prepare(x, segment_pos, attn_mask, w_q, w_kv, w_out, q_scale, k_scale):
    x2 = np.ascontiguousarray(np.asarray(x, np.float32).reshape(T, D))
    pos = np.asarray(segment_pos).reshape(T).astype(np.int64)
    am = np.asarray(attn_mask).reshape(T, T).astype(bool)

    # rope tables, fp32 like the reference
    half = H // 2
    fraction = (2.0 * np.arange(half, dtype=np.float32) / np.float32(H)).astype(np.float32)
    timescale = (np.float32(ROPE_BASE) ** fraction).astype(np.float32)
    sinusoid = (pos.astype(np.float32)[:, None] / timescale[None, :]) / np.float32(ROPE_SCALE)
    sin = np.sin(sinusoid).astype(np.float32)
    cos = np.cos(sinusoid).astype(np.float32)
    qsf = (1.0 + np.asarray(q_scale, np.float32))
    ksf = (1.0 + np.asarray(k_scale, np.float32))
    # tabs[t, 0..7, i]: q: cos*qsf_f, sin*qsf_s, cos*qsf_s, sin*qsf_f; then k
    tabs = np.empty((T, 8, half), np.float32)
    tabs[:, 0] = cos * qsf[None, :half]
    tabs[:, 1] = sin * qsf[None, half:]
    tabs[:, 2] = cos * qsf[None, half:]
    tabs[:, 3] = sin * qsf[None, :half]
    tabs[:, 4] = cos * ksf[None, :half]
    tabs[:, 5] = sin * ksf[None, half:]
    tabs[:, 6] = cos * ksf[None, half:]
    tabs[:, 7] = sin * ksf[None, :half]
    tabs = np.ascontiguousarray(tabs.reshape(TB, 128, 8, half))

    # combined mask -> band structure + additive mask tiles (transposed [k, q])
    sliding = (pos[None, :] > pos[:, None] - WINDOW) & (pos[None, :] < pos[:, None] + WINDOW)
    comb = am & sliding
    band = []
    mask_list = []
    for j in range(JQ):
        row = []
        sub_q = comb[j * 512:(j + 1) * 512]
        for kb in range(T // 128):
            sub = sub_q[:, kb * 128:(kb + 1) * 128]
            if not sub.any():
                continue
            if sub.all():
                row.append((kb, None))
            else:
                mask_list.append(
                    np.where(sub.T, np.float32(0.0), np.float32(K_MASK)))
                row.append((kb, len(mask_list) - 1))
        band.append(row)
    masks = (np.ascontiguousarray(np.stack(mask_list, axis=1).astype(np.float32))
             if mask_list else np.zeros((128, 1, 512), np.float32))

    # x transposed + tiled: xt[tb, p, dc, t] = x2[tb*128+t, dc*128+p]
    xt = np.ascontiguousarray(
        x2.reshape(TB, 128, DC, 128).transpose(0, 3, 2, 1))

    return x2, xt, tabs, band, masks


def kernel(x, segment_pos, attn_mask, w_q, w_kv, w_out, q_scale, k_scale):
    x = np.asarray(x, np.float32)
    w_q = np.asarray(w_q, np.float32)
    w_kv = np.asarray(w_kv, np.float32)
    w_out = np.asarray(w_out, np.float32)
    assert x.shape == (B, T, D) and w_q.shape == (N, D, H)

    x2, xt, tabs, band, masks = _host_prepare(
        x, segment_pos, attn_mask, w_q, w_kv, w_out, q_scale, k_scale)

    band_key = tuple(tuple(row) for row in band)
    if band_key not in _PROG_CACHE:
        _PROG_CACHE[band_key] = _build_program(band_key, band)
    nc = _PROG_CACHE[band_key]

    in_maps = []
    for c in range(NCORES):
        wqc = np.concatenate([w_q[2 * c], w_q[2 * c + 1]], axis=1)  # [D, 512]
        wqc = np.ascontiguousarray(
            wqc.reshape(DC // 4, 4, 128, 512).transpose(0, 2, 1, 3))
        wkvc = np.concatenate([w_kv[0, c], w_kv[1, c]], axis=1)     # [D, 512]
        wkvc = np.ascontiguousarray(
            wkvc.reshape(DC // 4, 4, 128, 512).transpose(0, 2, 1, 3))
        # wo[dch, p, hh, n] = w_out[2c + head][hc*128 + p, dch*512 + n]
        woc = np.empty((DCH, 128, 4, 512), np.float32)
        for hh in range(4):
            head, hc = hh >> 1, hh & 1
            woc[:, :, hh, :] = w_out[2 * c + head][hc * 128:(hc + 1) * 128] \
                .reshape(128, DCH, 512).transpose(1, 0, 2)
        in_maps.append({
            "xt": xt, "wq": wqc, "wkv": wkvc, "wo": np.ascontiguousarray(woc),
            "tabs": tabs, "masks": masks,
            "onec": np.ones((128, 1), ml_dtypes.bfloat16),
            "oner": np.ones((1, 128), np.float32),
        })

    trace = bool(int(os.environ.get("BASS_ATTN_TRACE", "0")))
    res = run_bass_kernel_spmd(nc, in_maps, list(range(NCORES)), trace=trace)
    if trace and res.exec_time_ns is not None:
        print(f"HW exec time: {res.exec_time_ns} ns")
        kernel._last_exec_ns = res.exec_time_ns
        kernel._last_results = res

    total = np.zeros((T, D), np.float64)
    for c in range(NCORES):
        total += res.results[c]["out"].astype(np.float64)
    return total.astype(np.float32).reshape(B, T, D)


# revision 46
# speedup vs baseline: 1.0330x; 1.0100x over previous
"""Trainium2 Bass kernel for nn_Attention_16441134809282 (sparse sliding-window GQA).

Self-contained: hardcodes shapes from the problem spec.
Sharding: 8 cores; core c owns q-heads {2c, 2c+1} and kv-head c (tensor
parallel over heads). Each core computes a partial output [T, D] (its heads'
contribution through w_out); the host sums the 8 partials.

Matmuls run in float32r (TRN2 fast fp32 path, 1 cyc/row at free-dim >= 256)
with fp32 PSUM accumulation; the value path (V, exp-probs) runs in bf16.

V1 schedule (vs. prior baseline at ~435 us):
- phase 1 warm start: q-proj runs one tb ahead of kv-proj so the wkv weight
  DMA wait hides under q matmuls; wq group 0 rides the sync ring first so
  the first matmul starts ~5 us earlier.
- phase 2: AV flush lag 2 (deeper PE/ACT pipeline), full tiles before masked
  tiles per j (mask DMA prefetch window), mask loads prefetched one j ahead,
  the per-head rep matmul deferred past the next head's first QK, and a
  global lazy out-projection unit queue (units pop inside attention, double
  on unmasked tiles, drain with ACT copies).
- per-unit [128,512] output stores (earlier final DMA).
"""
import os
from collections import deque

import ml_dtypes
import numpy as np

import concourse.bass as bass  # noqa: F401
import concourse.mybir as mybir
import concourse.tile as tile
from concourse import bacc
from concourse.bass_utils import run_bass_kernel_spmd
from concourse.masks import make_identity

# problem constants
B, T, D = 1, 2048, 3072
N, K, H = 16, 8, 256
G = N // K
SOFT_CAP = 50.0
WINDOW = 1024
ROPE_BASE = 10000.0
ROPE_SCALE = 1.0
K_MASK = -2.3819763e38
EPS = 1e-6

NCORES = 8
TB = T // 128       # 16 t-blocks
DC = D // 128       # 24 d-chunks (contraction)
JP = T // 256       # 8 query chunks of 256 (attention granularity)
DCH = D // 512      # 6 output d-chunks of 512

F32 = mybir.dt.float32
F32R = mybir.dt.float32r
BF16 = mybir.dt.bfloat16
AF = mybir.ActivationFunctionType
ALU = mybir.AluOpType

_PROG_CACHE: dict = {}


def _build_program(band_key, band, debug=False):
    """band: list (len JQ) of list of (kb, mask_slot or None)."""
    n_masks = max(1, sum(1 for row in band for (_, m) in row if m is not None))
    nc = bacc.Bacc("TRN2", target_bir_lowering=False, debug=False, num_devices=NCORES)

    xt_e = nc.dram_tensor("xt", [TB, 128, DC, 128], F32R, kind="ExternalInput").ap()
    wq_e = nc.dram_tensor("wq", [DC // 4, 128, 4, 512], F32R, kind="ExternalInput").ap()
    wkv_e = nc.dram_tensor("wkv", [DC // 4, 128, 4, 512], F32R, kind="ExternalInput").ap()
    wo_e = nc.dram_tensor("wo", [DCH, 128, 4, 512], BF16, kind="ExternalInput").ap()
    tabs_e = nc.dram_tensor("tabs", [TB, 128, 8, 128], F32, kind="ExternalInput").ap()
    masks_e = nc.dram_tensor("masks", [128, n_masks, 256], F32, kind="ExternalInput").ap()
    onec_e = nc.dram_tensor("onec", [128, 1], BF16, kind="ExternalInput").ap()
    oner_e = nc.dram_tensor("oner", [1, 128], F32R, kind="ExternalInput").ap()
    out_e = nc.dram_tensor("out", [T, D], F32, kind="ExternalOutput").ap()
    dbg = {}
    if debug:
        dbg["qt"] = nc.dram_tensor("dbg_qt", [128, 4 * TB, 128], F32R,
                                   kind="ExternalOutput").ap()
        dbg["kt"] = nc.dram_tensor("dbg_kt", [128, 2 * TB, 128], F32R,
                                   kind="ExternalOutput").ap()
        dbg["v"] = nc.dram_tensor("dbg_v", [128, TB, 256], BF16,
                                  kind="ExternalOutput").ap()
        dbg["enc"] = nc.dram_tensor("dbg_enc", [128, 32, 256], BF16,
                                    kind="ExternalOutput").ap()

    with tile.TileContext(nc) as tc:
        with (
            tc.tile_pool(name="pers", bufs=1) as pers,
            # logits psum lives below phase 1's pools, so the first phase-2
            # QK matmuls have no WAR wait on phase-1 psum release
            tc.tile_pool(name="plg", bufs=2, space="PSUM") as plgp,
        ):
            # persistent SBUF: transposed Q/K (f32r), natural V (bf16)
            QT = pers.tile([128, 4 * TB, 128], F32R)   # chunk = tb*4 + head*2 + hc
            KT = pers.tile([128, 2 * TB, 128], F32R)   # chunk = tb*2 + hc
            V = pers.tile([128, TB, 256], BF16)        # [t%128, tb, h]

            QTv = QT[:].rearrange("p (tb hh) f -> p hh tb f", hh=4)
            KTv = KT[:].rearrange("p (tb hc) f -> p hc tb f", hc=2)

            # ---------------- Phase 1: projections + RMSNorm + RoPE + transpose
            with (
                tc.tile_pool(name="wts", bufs=1) as wts,
                tc.tile_pool(name="xs", bufs=3) as xsp,
                tc.tile_pool(name="tab", bufs=2) as tabp,
                tc.tile_pool(name="rot", bufs=2) as rotp,
                tc.tile_pool(name="wk", bufs=1) as wk,
                tc.tile_pool(name="psq", bufs=2, space="PSUM") as psqp,
                tc.tile_pool(name="pskv", bufs=2, space="PSUM") as pskvp,
                tc.tile_pool(name="ptrq", bufs=1, space="PSUM") as ptrqp,
                tc.tile_pool(name="ptrk", bufs=1, space="PSUM") as ptrkp,
            ):
                # group 0 split in two so the very first matmul waits on a
                # 0.5 MB transfer instead of 1 MB
                wq0a = wts.tile([128, 2, 512], F32R, name="wq0a")
                wq0b = wts.tile([128, 2, 512], F32R, name="wq0b")
                wq_c = [None] + [wts.tile([128, 4, 512], F32R, tag=f"wq{g}",
                                          name=f"wq{g}")
                                 for g in range(1, DC // 4)]
                wkv_c = [wts.tile([128, 4, 512], F32R, tag=f"wkv{g}", name=f"wkv{g}")
                         for g in range(DC // 4)]

                def wq_at(dc):
                    if dc < 2:
                        return wq0a[:, dc, :]
                    if dc < 4:
                        return wq0b[:, dc - 2, :]
                    return wq_c[dc // 4][:, dc % 4, :]

                xs_t, tabq_t, tabk_t = {}, {}, {}

                def load_x(tb):
                    xs = xsp.tile([128, DC, 128], F32R, tag="xs", name=f"xs{tb}")
                    nc.sync.dma_start(xs[:, 0:DC // 2, :], xt_e[tb, :, 0:DC // 2, :])
                    nc.sync.dma_start(xs[:, DC // 2:, :], xt_e[tb, :, DC // 2:, :])
                    xs_t[tb] = xs

                def load_tabq(tb):
                    tq = tabp.tile([128, 4, 128], F32, tag="tabq",
                                   name=f"tabq{tb}")
                    nc.sync.dma_start(tq[:], tabs_e[tb, :, 0:4, :])
                    tabq_t[tb] = tq

                def load_tabk(tb):
                    # scalar ring: it drains after wq1..5, well before the
                    # sync ring works through the wkv groups + xs strips
                    tk = tabp.tile([128, 4, 128], F32, tag="tabk", name=f"tabk{tb}")
                    nc.scalar.dma_start(tk[:], tabs_e[tb, :, 4:8, :])
                    tabk_t[tb] = tk

                # DMA priority: wq0 rides the sync ring first (earliest first
                # matmul); all remaining weight groups stream on the scalar
                # ring in consumption order while the sync ring carries the
                # x strips + q-tables.
                nc.sync.dma_start(wq0a[:], wq_e[0, :, 0:2, :])
                nc.sync.dma_start(wq0b[:], wq_e[0, :, 2:4, :])
                for g in range(1, DC // 4):
                    nc.scalar.dma_start(wq_c[g][:], wq_e[g])
                for g in range(DC // 4):
                    nc.scalar.dma_start(wkv_c[g][:], wkv_e[g])
                load_x(0)
                load_x(1)
                load_x(2)
                load_tabq(0)
                load_tabq(1)

                ident = wts.tile([128, 128], F32)
                make_identity(nc, ident[:])
                eps_t = wts.tile([128, 1], F32)
                nc.gpsimd.memset(eps_t[:], EPS)
                # touch ACT tables once now so loads don't land on the
                # phase-2 critical path
                warm = wts.tile([128, 1], F32)
                nc.scalar.activation(warm[:], eps_t[:], AF.Tanh)
                nc.scalar.activation(warm[:], warm[:], AF.Exp)

                def rsqrt_of_meansq(src_ap, nfree, tag):
                    """rs = rsqrt(mean(src^2) + EPS), per partition row."""
                    scr = wk.tile([128, nfree], F32, tag="sq_scr")
                    ssq = wk.tile([128, 1], F32, tag=tag + "_ssq")
                    nc.scalar.activation(scr[:], src_ap, AF.Square, accum_out=ssq[:])
                    sd = wk.tile([128, 1], F32, tag=tag + "_sd")
                    nc.scalar.activation(sd[:], ssq[:], AF.Sqrt,
                                         scale=1.0 / nfree, bias=eps_t[:])
                    rs = wk.tile([128, 1], F32, tag=tag + "_rs")
                    nc.vector.reciprocal(rs[:], sd[:])
                    return rs

                def rope_norm(dst, psrc, off, rs, tabs):
                    """dst[:, off:off+256] = rope(psrc[:, off:off+256] * rs), with
                    (1+scale) folded into tabs[0..3]."""
                    f = psrc[:, off:off + 128]
                    s = psrc[:, off + 128:off + 256]
                    dst_f = dst[:, off:off + 128]
                    dst_s = dst[:, off + 128:off + 256]
                    t2 = wk.tile([128, 128], F32, tag="rope_t2", bufs=2)
                    nc.vector.scalar_tensor_tensor(
                        dst_f, f, rs[:], tabs[:, 0, :], ALU.mult, ALU.mult)
                    nc.vector.scalar_tensor_tensor(
                        t2[:], s, rs[:], tabs[:, 1, :], ALU.mult, ALU.mult)
                    nc.vector.tensor_sub(dst_f, dst_f, t2[:])
                    nc.vector.scalar_tensor_tensor(
                        dst_s, s, rs[:], tabs[:, 2, :], ALU.mult, ALU.mult)
                    nc.vector.scalar_tensor_tensor(
                        t2[:], f, rs[:], tabs[:, 3, :], ALU.mult, ALU.mult)
                    nc.vector.tensor_add(dst_s, dst_s, t2[:])

                qrot_t = {}
                pend = None  # (tb, qrot, krot) awaiting transpose

                def do_q(tb):
                    xs = xs_t[tb]
                    psq = psqp.tile([128, 512], F32, tag="psq")
                    for dc in range(DC):
                        nc.tensor.matmul(psq[:], xs[:, dc, :], wq_at(dc),
                                         start=(dc == 0), stop=(dc == DC - 1))
                    qrot = rotp.tile([128, 512], F32, tag="qrot", bufs=3,
                                     name=f"qrot{tb}")
                    for head in range(2):
                        rs = rsqrt_of_meansq(psq[:, head * 256:(head + 1) * 256],
                                             256, f"q{head}")
                        rope_norm(qrot, psq, head * 256, rs, tabq_t[tb])
                    qrot_t[tb] = qrot
                    # tabq ring is 2 deep; tb's epilogue above was the last
                    # reader of the slot tb+2 recycles
                    if tb + 2 < TB:
                        load_tabq(tb + 2)

                def do_kv(tb):
                    nonlocal pend
                    load_tabk(tb)
                    if pend is not None:
                        _emit_transposes(nc, tc, pend, ident, QT, KT,
                                         ptrqp, ptrkp)
                    xs = xs_t[tb]
                    pskv = pskvp.tile([128, 512], F32, tag="pskv")
                    for dc in range(DC):
                        nc.tensor.matmul(pskv[:], xs[:, dc, :],
                                         wkv_c[dc // 4][:, dc % 4, :],
                                         start=(dc == 0), stop=(dc == DC - 1))
                    krot = rotp.tile([128, 256], F32, tag="krot", name=f"krot{tb}")
                    rs = rsqrt_of_meansq(pskv[:, 0:256], 256, "k")
                    rope_norm(krot, pskv, 0, rs, tabk_t[tb])
                    rs = rsqrt_of_meansq(pskv[:, 256:512], 256, "v")
                    nc.vector.tensor_scalar_mul(V[:, tb, :], pskv[:, 256:512], rs[:])
                    pend = (tb, qrot_t[tb], krot)
                    # prefetch the x strip this buffer slot frees up (kv(tb)'s
                    # matmuls above are the last readers of xs[tb], so the WAR
                    # on the recycled buffer is correct and already emitted)
                    if tb + 3 < TB:
                        load_x(tb + 3)

                # warm-start order: q runs ahead of kv so the wkv weight
                # stream (behind wq on the scalar ring) hides under q matmuls
                do_q(0)
                do_q(1)
                do_q(2)
                do_kv(0)
                do_kv(1)
                for t in range(3, TB):
                    do_q(t)
                    do_kv(t - 1)
                do_kv(TB - 1)
                _emit_transposes(nc, tc, pend, ident, QT, KT, ptrqp, ptrkp)

            # ---------------- Phase 2+3: attention + output projection
            with tc.tile_pool(name="enc", bufs=1) as encpool:
                # chunk = head*16 + hc*8 + jp
                ENC = encpool.tile([128, 32, 256], BF16)
                ones_c = encpool.tile([128, 1], BF16)
                ones_r = encpool.tile([1, 128], F32R)
                nc.sync.dma_start(ones_c[:], onec_e[:])
                nc.sync.dma_start(ones_r[:], oner_e[:])
                wo_t = [encpool.tile([128, 4, 512], BF16, name=f"wo{dch}")
                        for dch in range(DCH)]
                if debug:
                    nc.sync.dma_start(dbg["qt"][:], QT[:])
                    nc.sync.dma_start(dbg["kt"][:], KT[:])
                    nc.sync.dma_start(dbg["v"][:], V[:])
                _phase2(nc, tc, band, QTv, KTv, V, ENC, ones_c, ones_r, masks_e,
                        wo_t, wo_e, out_e, plgp)
                if debug:
                    nc.sync.dma_start(dbg["enc"][:], ENC[:])

    nc.compile()
    return nc


def _phase2(nc, tc, band, QTv, KTv, V, ENC, ones_c, ones_r, masks_e,
            wo_t, wo_e, out_e, plgp):
    with (
        tc.tile_pool(name="mks", bufs=2) as mkp,
        tc.tile_pool(name="act", bufs=2) as actp,
        tc.tile_pool(name="sml", bufs=2) as sml,
        tc.tile_pool(name="ost", bufs=4) as ostp,
        tc.tile_pool(name="pen", bufs=1, space="PSUM") as penp,
        tc.tile_pool(name="pdn", bufs=1, space="PSUM") as pdnp,
        tc.tile_pool(name="prp", bufs=1, space="PSUM") as prpp,
        tc.tile_pool(name="pso", bufs=2, space="PSUM") as psop,
    ):
        # per-jp kb order: full tiles first (masked tiles get their DMA window)
        rows = [sorted(band[jp], key=lambda t: t[1] is not None)
                for jp in range(JP)]
        mtiles = {}  # jp -> {mslot: ap}

        def load_masks(jp):
            mslots = sorted(m for (_, m) in band[jp] if m is not None)
            if not mslots:
                mtiles[jp] = {}
                return
            m0, nm = mslots[0], len(mslots)
            assert mslots == list(range(m0, m0 + nm))
            mk = mkp.tile([128, nm, 256], F32, tag="mk", name=f"mkj{jp}")
            nc.sync.dma_start(mk[:], masks_e[:, m0:m0 + nm, :])
            mtiles[jp] = {m: mk[:, i, :] for i, m in enumerate(mslots)}

        jorder = [1, 0] + list(range(2, JP))
        load_masks(jorder[0])
        load_masks(jorder[1])

        pending = deque()
        drain_mode = False
        copy_tick = 0

        def unit(jp, r, dch):
            def emit():
                nonlocal copy_tick
                stage = ostp.tile([128, 512], F32, tag="stage",
                                  name=f"st{jp}_{r}_{dch}")
                po = psop.tile([128, 512], F32, tag="po")
                for hh in range(4):
                    head, hc = hh >> 1, hh & 1
                    nc.tensor.matmul(
                        po[:],
                        ENC[:, head * 16 + hc * 8 + jp, r * 128:(r + 1) * 128],
                        wo_t[dch][:, hh, :],
                        start=(hh == 0), stop=(hh == 3))
                # alternate the psum-drain copy between ACT and DVE so neither
                # engine's queue gates the po psum reuse
                copy_tick += 1
                if drain_mode or copy_tick % 2:
                    nc.scalar.activation(stage[:], po[:], AF.Copy)
                else:
                    nc.vector.tensor_copy(stage[:], po[:])
                tb = 2 * jp + r
                nc.sync.dma_start(
                    out_e[tb * 128:(tb + 1) * 128, dch * 512:(dch + 1) * 512],
                    stage[:])
            return emit

        fin_prev = None  # deferred per-head finalize (rep matmul + folds)
        first = True
        for jidx, jp in enumerate(jorder):
            kbs = rows[jp]
            nkb = len(kbs)
            for head in range(2):
                # each hc half padded to a full psum bank: the two AV
                # accumulation groups are pending simultaneously and a bank
                # is the zero-region granularity of matmul start=True
                enc_ps = penp.tile([128, 2, 512], F32, tag="enc")
                den_ps = pdnp.tile([1, 256], F32, tag="den")
                pend_av = deque()

                def flush_av():
                    i, kb, ex = pend_av.popleft()
                    nc.tensor.matmul(den_ps[:], ones_c[:], ex[:],
                                     start=(i == 0), stop=(i == nkb - 1))
                    for hc in range(2):
                        nc.tensor.matmul(
                            enc_ps[:, hc, 0:256],
                            V[:, kb, hc * 128:(hc + 1) * 128], ex[:],
                            start=(i == 0), stop=(i == nkb - 1))
                    if pending:
                        pending.popleft()()
                    if kbs[i][1] is None and pending:
                        pending.popleft()()

                for i, (kb, mslot) in enumerate(kbs):
                    lg = plgp.tile([128, 256], F32, tag="lg")
                    for hc in range(2):
                        nc.tensor.matmul(
                            lg[:], KTv[:, hc, kb, :],
                            QTv[:, head * 2 + hc, 2 * jp:2 * jp + 2, :],
                            start=(hc == 0), stop=(hc == 1))
                    th = actp.tile([128, 256], F32, tag="th")
                    nc.scalar.activation(th[:], lg[:], AF.Tanh,
                                         scale=1.0 / SOFT_CAP)
                    ex = actp.tile([128, 256], BF16, tag="ex", bufs=3)
                    if mslot is not None:
                        nc.vector.scalar_tensor_tensor(
                            th[:], th[:], SOFT_CAP, mtiles[jp][mslot],
                            ALU.mult, ALU.add)
                        nc.scalar.activation(ex[:], th[:], AF.Exp)
                    else:
                        nc.scalar.activation(ex[:], th[:], AF.Exp,
                                             scale=SOFT_CAP)
                    if i == 0 and fin_prev is not None:
                        fin_prev()
                        fin_prev = None
                    pend_av.append((i, kb, ex))
                    if len(pend_av) >= 3:
                        flush_av()
                while pend_av:
                    flush_av()

                # part A now (DVE copy of the denominator row); the PE rep
                # matmul + folds are deferred past the next head's first QK
                den_sb = sml.tile([1, 256], F32R, tag="den_sb")
                nc.vector.tensor_copy(den_sb[:], den_ps[:])

                def make_fin(jp=jp, head=head, den_sb=den_sb, enc_ps=enc_ps):
                    def fin():
                        rep_ps = prpp.tile([128, 256], F32, tag="rep")
                        nc.tensor.matmul(rep_ps[:], ones_r[:], den_sb[:],
                                         start=True, stop=True)
                        rep_rec = sml.tile([128, 256], F32, tag="rep_rec")
                        nc.vector.reciprocal_approx_fast(rep_rec[:], rep_ps[:])
                        for hc in range(2):
                            nc.vector.tensor_mul(
                                ENC[:, head * 16 + hc * 8 + jp, :],
                                enc_ps[:, hc, 0:256], rep_rec[:])
                        if head == 1:
                            pending.extend(unit(jp, r, dch)
                                           for dch in range(DCH)
                                           for r in range(2))
                    return fin

                fin_prev = make_fin()

            if first:
                for dch in range(DCH):
                    nc.scalar.dma_start(wo_t[dch][:], wo_e[dch])
                first = False
            if jidx + 2 < JP:
                load_masks(jorder[jidx + 2])

        fin_prev()
        drain_mode = True
        while pending:
            pending.popleft()()


def _emit_transposes(nc, tc, pend, ident, QT, KT, ptrqp, ptrkp):
    tb, qrot, krot = pend
    ptr = ptrqp.tile([128, 4, 128], F32, tag="ptrq")
    for c in range(4):
        nc.tensor.transpose(ptr[:, c, :], qrot[:, c * 128:(c + 1) * 128], ident[:])
    nc.scalar.activation(QT[:, tb * 4:tb * 4 + 4, :], ptr[:], AF.Copy)
    ptr2 = ptrkp.tile([128, 2, 128], F32, tag="ptrk")
    for c in range(2):
        nc.tensor.transpose(ptr2[:, c, :], krot[:, c * 128:(c + 1) * 128], ident[:])
    nc.vector.tensor_copy(KT[:, tb * 2:tb * 2 + 2, :], ptr2[:])


def _host_prepare(x, segment_pos, attn_mask, w_q, w_kv, w_out, q_scale, k_scale):
    x2 = np.ascontiguousarray(np.asarray(x, np.float32).reshape(T, D))
    pos = np.asarray(segment_pos).reshape(T).astype(np.int64)
    am = np.asarray(attn_mask).reshape(T, T).astype(bool)

    # rope tables, fp32 like the reference
    half = H // 2
    fraction = (2.0 * np.arange(half, dtype=np.float32) / np.float32(H)).astype(np.float32)
    timescale = (np.float32(ROPE_BASE) ** fraction).astype(np.float32)
    sinusoid = (pos.astype(np.float32)[:, None] / timescale[None, :]) / np.float32(ROPE_SCALE)
    sin = np.sin(sinusoid).astype(np.float32)
    cos = np.cos(sinusoid).astype(np.float32)
    qsf = (1.0 + np.asarray(q_scale, np.float32))
    ksf = (1.0 + np.asarray(k_scale, np.float32))
    # tabs[t, 0..7, i]: q: cos*qsf_f, sin*qsf_s, cos*qsf_s, sin*qsf_f; then k
    tabs = np.empty((T, 8, half), np.float32)
    tabs[:, 0] = cos * qsf[None, :half]
    tabs[:, 1] = sin * qsf[None, half:]
    tabs[:, 2] = cos * qsf[None, half:]
    tabs[:, 3] = sin * qsf[None, :half]
    tabs[:, 4] = cos * ksf[None, :half]
    tabs[:, 5] = sin * ksf[None, half:]
    tabs[:, 6] = cos * ksf[None, half:]
    tabs[:, 7] = sin * ksf[None, :half]
    tabs = np.ascontiguousarray(tabs.reshape(TB, 128, 8, half))

    # combined mask -> band structure + additive mask tiles (transposed [k, q])
    sliding = (pos[None, :] > pos[:, None] - WINDOW) & (pos[None, :] < pos[:, None] + WINDOW)
    comb = am & sliding
    band = []
    mask_list = []
    for jp in range(JP):
        row = []
        sub_q = comb[jp * 256:(jp + 1) * 256]
        for kb in range(T // 128):
            sub = sub_q[:, kb * 128:(kb + 1) * 128]
            if not sub.any():
                continue
            if sub.all():
                row.append((kb, None))
            else:
                mask_list.append(
                    np.where(sub.T, np.float32(0.0), np.float32(K_MASK)))
                row.append((kb, len(mask_list) - 1))
        band.append(row)
    masks = (np.ascontiguousarray(np.stack(mask_list, axis=1).astype(np.float32))
             if mask_list else np.zeros((128, 1, 256), np.float32))

    # x transposed + tiled: xt[tb, p, dc, t] = x2[tb*128+t, dc*128+p]
    xt = np.ascontiguousarray(
        x2.reshape(TB, 128, DC, 128).transpose(0, 3, 2, 1))

    return x2, xt, tabs, band, masks


def kernel(x, segment_pos, attn_mask, w_q, w_kv, w_out, q_scale, k_scale):
    x = np.asarray(x, np.float32)
    w_q = np.asarray(w_q, np.float32)
    w_kv = np.asarray(w_kv, np.float32)
    w_out = np.asarray(w_out, np.float32)
    assert x.shape == (B, T, D) and w_q.shape == (N, D, H)

    x2, xt, tabs, band, masks = _host_prepare(
        x, segment_pos, attn_mask, w_q, w_kv, w_out, q_scale, k_scale)

    band_key = tuple(tuple(row) for row in band)
    debug = bool(int(os.environ.get("BASS_ATTN_DEBUG", "0")))
    cache_key = (band_key, debug)
    if cache_key not in _PROG_CACHE:
        _PROG_CACHE[cache_key] = _build_program(band_key, band, debug=debug)
    nc = _PROG_CACHE[cache_key]

    in_maps = []
    for c in range(NCORES):
        wqc = np.concatenate([w_q[2 * c], w_q[2 * c + 1]], axis=1)  # [D, 512]
        wqc = np.ascontiguousarray(
            wqc.reshape(DC // 4, 4, 128, 512).transpose(0, 2, 1, 3))
        wkvc = np.concatenate([w_kv[0, c], w_kv[1, c]], axis=1)     # [D, 512]
        wkvc = np.ascontiguousarray(
            wkvc.reshape(DC // 4, 4, 128, 512).transpose(0, 2, 1, 3))
        # wo[dch, p, hh, n] = w_out[2c + head][hc*128 + p, dch*512 + n]
        woc = np.empty((DCH, 128, 4, 512), np.float32)
        for hh in range(4):
            head, hc = hh >> 1, hh & 1
            woc[:, :, hh, :] = w_out[2 * c + head][hc * 128:(hc + 1) * 128] \
                .reshape(128, DCH, 512).transpose(1, 0, 2)
        in_maps.append({
            "xt": xt, "wq": wqc, "wkv": wkvc,
            "wo": np.ascontiguousarray(woc).astype(ml_dtypes.bfloat16),
            "tabs": tabs, "masks": masks,
            "onec": np.ones((128, 1), ml_dtypes.bfloat16),
            "oner": np.ones((1, 128), np.float32),
        })

    trace = bool(int(os.environ.get("BASS_ATTN_TRACE", "0")))
    res = run_bass_kernel_spmd(nc, in_maps, list(range(NCORES)), trace=trace)
    if trace and res.exec_time_ns is not None:
        print(f"HW exec time: {res.exec_time_ns} ns")
        kernel._last_exec_ns = res.exec_time_ns
        kernel._last_results = res

    if debug:
        kernel._dbg_results = res.results
    total = np.zeros((T, D), np.float64)
    for c in range(NCORES):
        total += res.results[c]["out"].astype(np.float64)
    return total.astype(np.float32).reshape(B, T, D)
